# revision 3
# baseline (speedup 1.0000x reference)
"""CEHessianCalculator diagonal-Hessian kernel for 8 Trainium2 NeuronCores.

Math (reference):
    val     = x @ W.T + b                     [B, C]
    softmax = exp(val) / rowsum(exp(val))     [B, C]
    out     = mean_b(softmax @ W^2 - (softmax @ W)^2)   [D]

Device algorithm (C-sharded over 8 cores, b-chunked):
  Per core, with a local C-slice (C_LOC rows of W, padded):
    eb   = exp(b_local)                           (folds the bias: exp(v+b) = exp(v)*eb)
    WtT  = W_local.T            [D, C_LOC]        (PE transposes, resident in SBUF)
    W'   = W_local * eb[:,None] [C_LOC, D]        (resident)
    W''  = W_local^2 * eb[:,None]                 (resident)
    for each b-chunk of 512:
       v    = WtT.T-matmuls -> logits.T [c, b] per 128-c-tile (PSUM)
       ev   = exp(v)                               (ACT, no bias needed)
       U   += W'.T @ ev   (PSUM accumulate)        = (exp(val) @ W).T * ... pre-norm
       Q   += W''.T @ ev  (PSUM accumulate)
       s   += eb.T-weighted col-accumulation of ev (DVE fused mul-add)
    U, Q transposed to [b, d] and written with s to DRAM.
  ReduceScatter(add) over the 8 cores gives each core the full-C U, Q, s for
  its own 512-row b-slice; it finishes mean_b(Q/s - (U/s)^2) locally -> [D]
  partial sums; the host adds the 8 partials.

float32r matmuls (11-bit-mantissa operands, fp32 accumulate) keep the error
~1e-4 while running at full 1 cycle/row PE speed.
"""

import os
import numpy as np
from contextlib import ExitStack

import concourse.bass as bass
import concourse.bacc as bacc
import concourse.tile as tile
from concourse import mybir
from concourse.bass_utils import run_bass_kernel_spmd
from concourse.masks import make_identity

F32 = mybir.dt.float32
AFT = mybir.ActivationFunctionType
ALU = mybir.AluOpType

B, C, D = 4096, 50257, 128
NCORE = 8
T = 50                      # W tiles (of 128 rows) per core
C_LOC = T * 128             # 6400
C_PAD = NCORE * C_LOC       # 51200
NCHUNK = 8
CH = 512                    # b rows per chunk
B_PAD_VAL = -40.0           # exp(-40) ~ 4e-18: padded classes contribute nothing
MM_DT = mybir.dt.float32r


def _build():
    nc = bacc.Bacc("TRN2", target_bir_lowering=False, debug=False, num_devices=NCORE)
    x_d = nc.dram_tensor("x", [B, D], F32, kind="ExternalInput").ap()
    W_d = nc.dram_tensor("Wl", [C_LOC, D], F32, kind="ExternalInput").ap()
    b_d = nc.dram_tensor("bl", [C_LOC], F32, kind="ExternalInput").ap()
    out_d = nc.dram_tensor("out", [D], F32, kind="ExternalOutput").ap()

    with tile.TileContext(nc) as tc, ExitStack() as ctx:
        const = ctx.enter_context(tc.tile_pool(name="const", bufs=1))
        wres = ctx.enter_context(tc.tile_pool(name="wres", bufs=1))
        wld = ctx.enter_context(tc.tile_pool(name="wld", bufs=4))
        sb = ctx.enter_context(tc.tile_pool(name="sb", bufs=3))
        fin = ctx.enter_context(tc.tile_pool(name="fin", bufs=1))
        pv = ctx.enter_context(tc.tile_pool(name="pv", bufs=2, space="PSUM"))
        pacc = ctx.enter_context(tc.tile_pool(name="pacc", bufs=1, space="PSUM"))
        pt = ctx.enter_context(tc.tile_pool(name="pt", bufs=2, space="PSUM"))
        dram = ctx.enter_context(tc.tile_pool(name="dram", bufs=1, space="DRAM"))

        ident = const.tile([128, 128], F32)
        make_identity(nc, ident[:])
        ones = const.tile([128, 1], F32)
        nc.gpsimd.memset(ones[:], 1.0)

        b_sb = const.tile([128, T], F32)
        nc.gpsimd.dma_start(b_sb[:], b_d.rearrange("(t c) -> c t", c=128))
        eb = const.tile([128, T], F32)
        nc.scalar.activation(eb[:], b_sb[:], AFT.Exp)

        WtT = wres.tile([128, C_LOC], MM_DT)   # [d, c_loc]
        Wp = wres.tile([128, C_LOC], MM_DT)    # [c(tile-part), d] per 128-col block
        W2p = wres.tile([128, C_LOC], MM_DT)
        xT = wres.tile([128, B], MM_DT)        # [d, b]

        # ---- prep: xT = x.T (PE transpose, 4 tiles per PSUM bank) ----
        for g in range(B // 512):
            pst = pt.tile([128, 512], F32, tag="pt")
            for j in range(4):
                bt = g * 4 + j
                xb = wld.tile([128, 128], F32, tag="xload")
                nc.gpsimd.dma_start(xb[:], x_d[bt * 128:(bt + 1) * 128, :])
                nc.tensor.transpose(pst[:, j * 128:(j + 1) * 128], xb[:], ident[:])
            nc.scalar.activation(xT[:, g * 512:(g + 1) * 512], pst[:], AFT.Copy)

        # ---- prep: W residents ----
        n_wg = (T + 3) // 4
        for g in range(n_wg):
            tg = min(4, T - g * 4)
            pst = pt.tile([128, 512], F32, tag="pt")
            for j in range(tg):
                t = g * 4 + j
                wt = wld.tile([128, 128], F32, tag="wload")
                nc.gpsimd.dma_start(wt[:], W_d[t * 128:(t + 1) * 128, :])
                nc.tensor.transpose(pst[:, j * 128:(j + 1) * 128], wt[:], ident[:])
                nc.vector.tensor_scalar_mul(
                    Wp[:, t * 128:(t + 1) * 128], wt[:], eb[:, t:t + 1])
                nc.vector.scalar_tensor_tensor(
                    W2p[:, t * 128:(t + 1) * 128], wt[:], eb[:, t:t + 1], wt[:],
                    op0=ALU.mult, op1=ALU.mult)
            nc.scalar.activation(
                WtT[:, g * 512:g * 512 + tg * 128], pst[:, :tg * 128], AFT.Copy)

        # ---- main: b-chunks ----
        U_dram = dram.tile([B, D], F32, tag="Ud")
        Q_dram = dram.tile([B, D], F32, tag="Qd")
        s_dram = dram.tile([B], F32, tag="sd")
        s_all = fin.tile([128, 4 * NCHUNK], F32, tag="sall")
        NP = T // 2

        for h in range(NCHUNK):
            U_ps = pacc.tile([128, CH], F32, tag="U")
            Q_ps = pacc.tile([128, CH], F32, tag="Q")
            s_acc = sb.tile([128, CH], F32, tag="sacc")
            xs = xT[:, h * CH:(h + 1) * CH]
            for p in range(NP):
                t0, t1 = 2 * p, 2 * p + 1
                psv = pv.tile([128, 2 * CH], F32, tag="v")
                nc.tensor.matmul(psv[:, 0:CH], WtT[:, t0 * 128:(t0 + 1) * 128],
                                 xs, start=True, stop=True)
                nc.tensor.matmul(psv[:, CH:2 * CH], WtT[:, t1 * 128:(t1 + 1) * 128],
                                 xs, start=True, stop=True)
                ev = sb.tile([128, 2 * CH], MM_DT, tag="ev")
                nc.scalar.activation(ev[:], psv[:], AFT.Exp)
                ev0 = ev[:, 0:CH]
                ev1 = ev[:, CH:2 * CH]
                nc.tensor.matmul(U_ps[:], Wp[:, t0 * 128:(t0 + 1) * 128], ev0,
                                 start=(p == 0), stop=False)
                nc.tensor.matmul(U_ps[:], Wp[:, t1 * 128:(t1 + 1) * 128], ev1,
                                 start=False, stop=(p == NP - 1))
                nc.tensor.matmul(Q_ps[:], W2p[:, t0 * 128:(t0 + 1) * 128], ev0,
                                 start=(p == 0), stop=False)
                nc.tensor.matmul(Q_ps[:], W2p[:, t1 * 128:(t1 + 1) * 128], ev1,
                                 start=False, stop=(p == NP - 1))
                ev0f = ev0.bitcast(F32)
                ev1f = ev1.bitcast(F32)
                if p == 0:
                    nc.vector.tensor_scalar_mul(s_acc[:], ev0f, eb[:, t0:t0 + 1])
                else:
                    nc.vector.scalar_tensor_tensor(
                        s_acc[:], ev0f, eb[:, t0:t0 + 1], s_acc[:],
                        op0=ALU.mult, op1=ALU.add)
                nc.vector.scalar_tensor_tensor(
                    s_acc[:], ev1f, eb[:, t1:t1 + 1], s_acc[:],
                    op0=ALU.mult, op1=ALU.add)

            # s: transpose c->free then reduce along free dim (keeps all DMAs
            # multi-partition; single-partition DMAs fail NEFF load)
            pss = pt.tile([128, CH], F32, tag="pt")
            for j in range(4):
                nc.tensor.transpose(pss[:, j * 128:(j + 1) * 128],
                                    s_acc[:, j * 128:(j + 1) * 128], ident[:])
            for j in range(4):
                nc.vector.tensor_reduce(
                    s_all[:, h * 4 + j:h * 4 + j + 1],
                    pss[:, j * 128:(j + 1) * 128],
                    axis=mybir.AxisListType.X, op=ALU.add)

            # U/Q: PSUM -> SBUF, transpose to [b, d], export
            for acc_ps, dst, tg in ((U_ps, U_dram, "u"), (Q_ps, Q_dram, "q")):
                a_sb = sb.tile([128, CH], F32, tag="acc_sb")
                nc.vector.tensor_copy(a_sb[:], acc_ps[:])
                ptt = pt.tile([128, CH], F32, tag="pt")
                for j in range(4):
                    nc.tensor.transpose(ptt[:, j * 128:(j + 1) * 128],
                                        a_sb[:, j * 128:(j + 1) * 128], ident[:])
                aT_sb = sb.tile([128, CH], F32, tag="accT_sb")
                nc.scalar.activation(aT_sb[:], ptt[:], AFT.Copy)
                nc.gpsimd.dma_start(
                    dst[h * CH:(h + 1) * CH, :].rearrange("(j p) d -> p j d", p=128),
                    aT_sb[:].rearrange("p (j d) -> p j d", d=128))

        nc.gpsimd.dma_start(
            s_dram[:].rearrange("(h j p) -> p (h j)", p=128, j=4), s_all[:])

        # ---- reduce across cores; each core finishes its own b-slice ----
        U_rs = dram.tile([CH, D], F32, tag="Urs")
        Q_rs = dram.tile([CH, D], F32, tag="Qrs")
        s_rs = dram.tile([CH], F32, tag="srs")
        groups = [list(range(NCORE))]
        nc.gpsimd.collective_compute(
            "ReduceScatter", ALU.add, replica_groups=groups,
            ins=[U_dram[:]], outs=[U_rs[:]])
        nc.gpsimd.collective_compute(
            "ReduceScatter", ALU.add, replica_groups=groups,
            ins=[Q_dram[:]], outs=[Q_rs[:]])
        nc.gpsimd.collective_compute(
            "ReduceScatter", ALU.add, replica_groups=groups,
            ins=[s_dram[:]], outs=[s_rs[:]])

        Urs_sb = fin.tile([128, CH], F32, tag="Ursb")
        nc.gpsimd.dma_start(Urs_sb[:].rearrange("p (j d) -> p j d", d=128),
                            U_rs[:].rearrange("(j p) d -> p j d", p=128))
        Qrs_sb = fin.tile([128, CH], F32, tag="Qrsb")
        nc.gpsimd.dma_start(Qrs_sb[:].rearrange("p (j d) -> p j d", d=128),
                            Q_rs[:].rearrange("(j p) d -> p j d", p=128))
        srs_sb = fin.tile([128, 4], F32, tag="srsb")
        nc.gpsimd.dma_start(srs_sb[:], s_rs[:].rearrange("(j p) -> p j", p=128))
        r_sb = fin.tile([128, 4], F32, tag="rsb")
        nc.vector.reciprocal(r_sb[:], srs_sb[:])

        res_ps = pt.tile([128, 1], F32, tag="pt")
        for j in range(4):
            t1 = fin.tile([128, 128], F32, tag="t1")
            nc.vector.tensor_scalar_mul(
                t1[:], Urs_sb[:, j * 128:(j + 1) * 128], r_sb[:, j:j + 1])
            t2 = fin.tile([128, 128], F32, tag="t2")
            nc.vector.tensor_mul(t2[:], t1[:], t1[:])
            t3 = fin.tile([128, 128], F32, tag="t3")
            nc.vector.scalar_tensor_tensor(
                t3[:], Qrs_sb[:, j * 128:(j + 1) * 128], r_sb[:, j:j + 1], t2[:],
                op0=ALU.mult, op1=ALU.subtract)
            # out = t3.T @ ones: sums over the 128 b-partitions -> [128 d, 1]
            nc.tensor.matmul(res_ps[:], t3[:], ones[:],
                             start=(j == 0), stop=(j == 3))
        res_sb = fin.tile([128, 1], F32, tag="res_sb")
        nc.scalar.activation(res_sb[:], res_ps[:], AFT.Copy, scale=1.0 / B)
        nc.gpsimd.dma_start(out_d[:].rearrange("(p one) -> p one", one=1), res_sb[:])

    nc.compile()
    return nc


_NC = None


def _get_nc():
    global _NC
    if _NC is None:
        _NC = _build()
    return _NC


def kernel(x, W, b, _trace=False, _trace_kwargs=None):
    x = np.ascontiguousarray(np.asarray(x, dtype=np.float32))
    W = np.asarray(W, dtype=np.float32)
    b = np.asarray(b, dtype=np.float32)
    assert x.shape == (B, D) and W.shape == (C, D) and b.shape == (C,)

    W_pad = np.zeros((C_PAD, D), dtype=np.float32)
    W_pad[:C] = W
    b_pad = np.full((C_PAD,), B_PAD_VAL, dtype=np.float32)
    b_pad[:C] = b

    in_maps = []
    for k in range(NCORE):
        in_maps.append({
            "x": x,
            "Wl": np.ascontiguousarray(W_pad[k * C_LOC:(k + 1) * C_LOC]),
            "bl": np.ascontiguousarray(b_pad[k * C_LOC:(k + 1) * C_LOC]),
        })

    nc = _get_nc()
    r = run_bass_kernel_spmd(
        nc, in_maps, list(range(NCORE)),
        trace=_trace, **(_trace_kwargs or {}))
    out = np.zeros((D,), dtype=np.float64)
    for k in range(NCORE):
        out += r.results[k]["out"].astype(np.float64)
    if _trace:
        return out.astype(np.float32), r
    return out.astype(np.float32)


if __name__ == "__main__":
    rng = np.random.default_rng(0)
    x = rng.standard_normal((B, D)).astype(np.float32)
    W = (0.01 * rng.standard_normal((C, D))).astype(np.float32)
    b = (0.01 * rng.standard_normal((C,))).astype(np.float32)
    got = kernel(x, W, b)
    # numpy reference
    val = x.astype(np.float64) @ W.astype(np.float64).T + b.astype(np.float64)
    e = np.exp(val)
    sm = e / e.sum(1, keepdims=True)
    ref = (sm @ (W.astype(np.float64) ** 2) - (sm @ W.astype(np.float64)) ** 2).mean(0)
    rel = np.abs(got - ref) / (np.abs(ref).max())
    print("scale-rel max err:", rel.max())
